# revision 30
# baseline (speedup 1.0000x reference)
"""DistanceLoss (EDT + weighted softmax loss) on 8 Trainium2 NeuronCores.

Sharding: data-parallel over batch. Each of the 8 cores processes 2 of the 16
batch samples (all 5 classes). Per (b, c) slice:

  Stage 1 (column distances g): the 1D distance along h is computed with a
  banded MATMUL log-sum trick on the idle PE array:
      S[i,j] = sum_{|s|<=8} 64^{-|s|} z[i+s, j]  (contract over partitions)
  so d_est = -log64(S + 64^-9) lies in (g-0.19, g] and with x = d_est+0.33
  both trunc(x) and round-nearest(x) equal g exactly for g <= 8 (saturating
  to 9 beyond, which preserves the band-check semantics) — one DVE f32->i32
  cast floors g regardless of the convert rounding mode.  Chain per class
  group: PE matmuls -> ACT Ln (from PSUM) -> ACT Identity (scale+bias) ->
  DVE casts -> ACT Square -> G2.

  Stage 2 (banded min-plus along w, radius R=6) in bf16 on DVE.  Classes are
  processed in two groups ({0,1} then {2,3,4}) so the DVE min-plus of group
  0 overlaps the PE/ACT stage-1 chain of group 1 (and of the next sample).

  Softmax pieces on ACT/DVE in bf16; per-class partial sums
  S1 = sum(d*probs), S2 = sum_present(probs) via DVE scalar_tensor_tensor
  accumulators; per-class max(d^2) via one 5-class tensor_reduce.

Host combine: loss = sum_{b,c} w_c/sum(w) * (S1 - sqrt(maxd2)*S2) / N.
Exact whenever true max EDT distance <= R (verified on gathered maxd2;
pure-numpy exact fallback otherwise — never taken for the target inputs).
"""

import math
import numpy as np

B, C, H, W = 16, 5, 256, 256
NCORES = 8
BPC = B // NCORES  # batches per core
R = 6              # min-plus band radius (exact iff max EDT distance <= R)
P = 128
CW = W + 6         # padded chunk width (pads >= R break min-plus chaining)
GBAND = 8          # log-sum band for column distances (exact g <= GBAND)
LSB = 64.0         # log-sum base (spread log_b(2b/(b-1)) ~ 0.17 << 1)
LNB = math.log(LSB)
XBIAS = 0.33       # trunc(d_est + XBIAS) == round(d_est + XBIAS) == g
CGROUPS = ((0,), (1, 2, 3, 4))

_CACHE = {}


def _host_wband():
    """Banded 64^{-|dist|} weight blocks for the column-distance matmul.
    [128, 3, 128] bf16: [:,0,:]=diag block, [:,1,:]=block(hb_in=0->hb_out=1),
    [:,2,:]=block(hb_in=1->hb_out=0). lhsT convention: [k=p_in, m=p_out].
    """
    idx = np.arange(P)
    d_diag = np.abs(idx[:, None] - idx[None, :]).astype(np.float64)
    d_01 = (P + idx[None, :] - idx[:, None]).astype(np.float64)  # |h_out-h_in|
    d_10 = (P + idx[:, None] - idx[None, :]).astype(np.float64)
    blocks = []
    for dm in (d_diag, d_01, d_10):
        w = np.where(dm <= GBAND, LSB ** (-dm), 0.0)
        blocks.append(w)
    out = np.stack(blocks, axis=1).astype(np.float32)  # [128, 3, 128]
    import ml_dtypes
    return out.astype(ml_dtypes.bfloat16)


def _build_nc(legalize=True, race_detect=True, walrus_fixups=True):
    import concourse.bass as bass
    import concourse.mybir as mybir
    import concourse.tile as tile

    f32 = mybir.dt.float32
    i32 = mybir.dt.int32
    bf16 = mybir.dt.bfloat16
    Alu = mybir.AluOpType
    Act = mybir.ActivationFunctionType

    nc = bass.Bass(detect_race_conditions=race_detect)
    pred_d = nc.dram_tensor("predictions", [BPC, C, H, W], f32, kind="ExternalInput")
    tgt_d = nc.dram_tensor("targets", [BPC, H, W], i32, kind="ExternalInput")
    wband_d = nc.dram_tensor("wband", [P, 3, P], bf16, kind="ExternalInput")
    # stats columns: [0:10] S1 (b*5+c), [10:20] S2, [20:30] maxd2, [30:32] pad
    out_d = nc.dram_tensor("out_stats", [P, 32], f32, kind="ExternalOutput")

    with tile.TileContext(nc) as tc:
        with (
            tc.tile_pool(name="const", bufs=1) as cpool,
            tc.tile_pool(name="work", bufs=2) as pool,
            tc.tile_pool(name="dmabuf", bufs=2) as dpool,
            tc.tile_pool(name="psA", bufs=1, space="PSUM") as psA,
            tc.tile_pool(name="psB", bufs=1, space="PSUM") as psB,
            tc.tile_pool(name="psC", bufs=1, space="PSUM") as psC,
        ):
            # hoist all DMAs: targets first (they gate the z/matmul chain)
            t_tiles = []
            pred_tiles = []
            for b in range(BPC):
                t_i32 = dpool.tile([P, 2, W], i32, tag="t_i32")
                nc.sync.dma_start(
                    t_i32[:], tgt_d[b].rearrange("(n p) w -> p n w", p=P))
                t_tiles.append(t_i32)
            wsb = cpool.tile([P, 3, P], bf16)
            nc.sync.dma_start(wsb[:], wband_d[:])
            for b in range(BPC):
                pred = dpool.tile([P, 2, C, W], f32, tag="pred")
                pred_v = pred_d[b].rearrange("c (n p) w -> p n c w", p=P)
                for hb in range(2):
                    nc.sync.dma_start(pred[:, hb], pred_v[:, hb])
                pred_tiles.append(pred)

            stats = cpool.tile([P, 32], f32)
            nc.vector.memset(stats[:], 0.0)
            ones1 = cpool.tile([P, 1], bf16)
            nc.vector.memset(ones1[:], 1.0)

            bias_ln = cpool.tile([P, 1], f32)
            nc.vector.memset(bias_ln[:], LSB ** -9)
            bias_x = cpool.tile([P, 1], f32)
            nc.vector.memset(bias_x[:], XBIAS)

            # warm-up during the DMA cold-start: preload the ln/exp ACT
            # table and give the PE some dummy matmuls to raise its pstate
            warm = cpool.tile([P, 64], bf16)
            nc.vector.memset(warm[:], 1.0)
            wjunk = cpool.tile([P, 64], f32)
            nc.scalar.activation(wjunk[:, :1], bias_ln[:], Act.Ln)
            psW = psC.tile([1, 64], f32, tag="psW")
            for _ in range(8):
                nc.tensor.matmul(psW[0:1, :], ones1[:], warm[:],
                                 start=True, stop=True)

            # ---- per-sample tiles (pool bufs=2 keeps both alive) ----
            zs, e_alls, eqs, G2bs, d2ps, d_alls = [], [], [], [], [], []
            for b in range(BPC):
                t_b = pool.tile([P, 2, W], bf16, tag="t_b")
                nc.vector.tensor_copy(t_b[:], t_tiles[b][:])
                z = pool.tile([P, C, 2, W], bf16, tag="z")
                for c in range(C):
                    nc.vector.tensor_scalar(
                        z[:, c], t_b[:], float(c), None, Alu.is_equal)
                zs.append(z)
                G2b = pool.tile([P, C, 2, CW], bf16, tag="G2b")
                nc.vector.memset(G2b[:, :, :, W:], 1.0e9)
                G2bs.append(G2b)
                d2ps.append(pool.tile([P, C, 2, CW], bf16, tag="d2p", name="d2p"))
                d_alls.append(pool.tile([P, C, 2, W], bf16, tag="d_all", name="d_all"))

            def stage1(b, gi):
                # column distances for class group gi of sample b:
                # banded matmul -> Ln -> scaled int cast -> Square -> G2
                grp = CGROUPS[gi]
                nclass = len(grp)
                c0 = grp[0]
                z = zs[b]
                ps = (psA if gi == 0 else psB)
                S = ps.tile([P, nclass, 2, W], f32, tag=f"S{gi}")
                for ci in range(nclass):
                    for hbo in range(2):
                        nc.tensor.matmul(
                            S[:, ci, hbo, :], wsb[:, 0, :],
                            z[:, grp[ci], hbo, :],
                            start=(hbo == 0), stop=False)
                for ci in range(nclass):
                    nc.tensor.matmul(
                        S[:, ci, 1, :], wsb[:, 1, :], z[:, grp[ci], 0, :],
                        start=False, stop=False)
                for ci in range(nclass):
                    nc.tensor.matmul(
                        S[:, ci, 0, :], wsb[:, 2, :], z[:, grp[ci], 1, :],
                        start=False, stop=True)
                NG = nclass * 2 * W
                u = pool.tile([P, 4, 2, W], f32, tag="u")
                uf = u[:].rearrange("p c a w -> p (c a w)")[:, :NG]
                nc.scalar.activation(
                    uf, S[:].rearrange("p c a w -> p (c a w)"),
                    Act.Ln, bias=bias_ln[:])
                g_i = pool.tile([P, 4, 2, W], i32, tag="g_i")
                gif = g_i[:].rearrange("p c a w -> p (c a w)")[:, :NG]
                nc.scalar.activation(
                    gif, uf, Act.Identity, scale=-1.0 / LNB, bias=bias_x[:])
                nc.scalar.activation(
                    G2bs[b][:, c0:c0 + nclass, :, :W],
                    gif.rearrange("p (c a w) -> p c a w", c=nclass, a=2, w=W),
                    Act.Square)

            def minplus(b, gi):
                # banded min-plus along w on the flat padded group slice
                grp = CGROUPS[gi]
                ncl = len(grp)
                c0 = grp[0]
                TLG = ncl * 2 * CW
                Gf = G2bs[b][:, c0:c0 + ncl].rearrange("p c a w -> p (c a w)")
                Df = d2ps[b][:, c0:c0 + ncl].rearrange("p c a w -> p (c a w)")
                nc.vector.tensor_copy(Df, Gf)
                for dlt in range(1, R + 1):
                    tmp = pool.tile([P, C, 2, CW], bf16, tag="tmp")
                    tmpf = tmp[:].rearrange("p c a w -> p (c a w)")[:, :TLG]
                    nc.vector.tensor_scalar(
                        tmpf, Gf, float(dlt * dlt), None, Alu.add)
                    nc.vector.tensor_tensor(
                        Df[:, dlt:], Df[:, dlt:], tmpf[:, :TLG - dlt],
                        Alu.min)
                    nc.vector.tensor_tensor(
                        Df[:, :TLG - dlt], Df[:, :TLG - dlt],
                        tmpf[:, dlt:], Alu.min)

            def sqrt_grp(b, gis):
                c0 = CGROUPS[gis[0]][0]
                ncl = sum(len(CGROUPS[gi]) for gi in gis)
                nc.scalar.activation(
                    d_alls[b][:, c0:c0 + ncl],
                    d2ps[b][:, c0:c0 + ncl, :, :W], Act.Sqrt)

            def softmax(b):
                # e = exp(pred), q = 1/sum_c e (ACT ln/exp), probs = e*q
                pred = pred_tiles[b]
                e_all = pool.tile([P, C, 2, W], bf16, tag="e_all")
                nc.scalar.activation(
                    e_all[:].rearrange("p c a w -> p a c w"), pred[:], Act.Exp)
                sA = pool.tile([P, 2, W], bf16, tag="sA")
                sB = pool.tile([P, 2, W], bf16, tag="sB")
                nc.vector.tensor_tensor(
                    sA[:], e_all[:, 0], e_all[:, 1], Alu.add)
                nc.vector.tensor_tensor(
                    sB[:], e_all[:, 2], e_all[:, 3], Alu.add)
                nc.vector.tensor_tensor(sA[:], sA[:], sB[:], Alu.add)
                nc.vector.tensor_tensor(sA[:], sA[:], e_all[:, 4], Alu.add)
                lg = pool.tile([P, 2, W], f32, tag="lg")
                nc.scalar.activation(lg[:], sA[:], Act.Ln)
                q = pool.tile([P, 2, W], bf16, tag="q")
                nc.scalar.activation(q[:], lg[:], Act.Exp, scale=-1.0)
                eq = pool.tile([P, C, 2, W], bf16, tag="eq")
                nc.vector.tensor_tensor(
                    eq[:], e_all[:],
                    q[:].unsqueeze(1).broadcast_to([P, C, 2, W]), Alu.mult)
                eqs.append(eq)

            def stats_grp(b, classes):
                # S1/S2 for the given classes: d*eq and z*eq products (DVE),
                # ones-matmul partition reduce (PE), free-axis accumulate
                # into stats row 0 (ACT)
                c0, ncl = classes[0], len(classes)
                eq = eqs[b]
                de = pool.tile([P, C, 2, W], bf16, tag="de")
                nc.vector.tensor_tensor(
                    de[:, c0:c0 + ncl].rearrange("p c a w -> p (c a w)"),
                    d_alls[b][:, c0:c0 + ncl].rearrange("p c a w -> p (c a w)"),
                    eq[:, c0:c0 + ncl].rearrange("p c a w -> p (c a w)"),
                    Alu.mult)
                ze = pool.tile([P, C, 2, W], bf16, tag="ze")
                nc.vector.tensor_tensor(
                    ze[:, c0:c0 + ncl].rearrange("p c a w -> p (c a w)"),
                    zs[b][:, c0:c0 + ncl].rearrange("p c a w -> p (c a w)"),
                    eq[:, c0:c0 + ncl].rearrange("p c a w -> p (c a w)"),
                    Alu.mult)
                junkA = pool.tile([1, 512], f32, tag="junkA")
                pairs = [(de, 0, c) for c in classes] + \
                        [(ze, 10, c) for c in classes]
                for k0 in range(0, len(pairs), 2):
                    psD = psC.tile([1, 2, 512], f32, tag="psD")
                    for j, (src_t, off, c) in enumerate(pairs[k0:k0 + 2]):
                        nc.tensor.matmul(
                            psD[0:1, j], ones1[:],
                            src_t[:, c].rearrange("p a w -> p (a w)"),
                            start=True, stop=True)
                        col = off + b * C + c
                        nc.scalar.activation(
                            junkA[0:1, :], psD[0:1, j], Act.Identity,
                            accum_out=stats[0:1, col:col + 1])

            def stats_dve(b, classes):
                # DVE stt accumulators (used for the final tail group so the
                # PE/ACT pipeline is not the last thing standing)
                eq = eqs[b]
                junk = pool.tile([P, 2, W], bf16, tag="junk")
                for c in classes:
                    c1 = b * C + c
                    nc.vector.scalar_tensor_tensor(
                        junk[:], d_alls[b][:, c], 0.0, eq[:, c],
                        Alu.add, Alu.mult,
                        accum_out=stats[:, c1:c1 + 1])
                    nc.vector.scalar_tensor_tensor(
                        junk[:], zs[b][:, c], 0.0, eq[:, c],
                        Alu.add, Alu.mult,
                        accum_out=stats[:, 10 + c1:11 + c1])

            def maxd2(b):
                nc.vector.tensor_reduce(
                    stats[:, 20 + b * C:20 + (b + 1) * C],
                    d2ps[b][:, :, :, :W], mybir.AxisListType.XY, Alu.max)

            # ---- schedule: interleave stage-1 and min-plus so ACT's
            # Square(G0) lands before anything queues behind it; b0 min-plus
            # overlaps b1 stage-1; the final (small) group's stats run as DVE
            # accumulators so the PE/ACT pipeline is not the tail ----
            stage1(0, 0)
            minplus(0, 0)
            stage1(0, 1)
            minplus(0, 1)
            stage1(1, 1)
            stage1(1, 0)
            softmax(0)
            sqrt_grp(0, (0, 1))
            maxd2(0)
            softmax(1)
            stats_grp(0, (0, 1, 2, 3, 4))
            minplus(1, 1)
            sqrt_grp(1, (1,))
            stats_grp(1, CGROUPS[1])
            minplus(1, 0)
            sqrt_grp(1, (0,))
            maxd2(1)
            stats_dve(1, CGROUPS[0])

            nc.sync.dma_start(out_d[:], stats[:])

    # walrus codegen in this toolchain allows only ONE sync wait per
    # instruction; split extras onto same-engine NoOps inserted right before
    # (engine streams are in-order, so this is semantically identical). It
    # also rejects the EVENT_SEMAPHORE_RANGE_CLEAR encoding; replace it with
    # per-semaphore `sem-wr-imm 0` updates on NoOps.
    if not walrus_fixups:
        return nc
    _walrus_fixups(nc)
    return nc


def _walrus_fixups(nc):
    import concourse.mybir as mybir
    rc_op = nc.isa.Opcode.NEURON_ISA_TPB_OPCODE_EVENT_SEMAPHORE_RANGE_CLEAR.value
    for f in nc.m.functions:
        for blk in f.blocks:
            newlist = []
            for inst in blk.instructions:
                si = inst.sync_info
                if si is not None and si.on_wait and len(si.on_wait) > 1:
                    for w in si.on_wait[:-1]:
                        newlist.append(mybir.InstNoOp(
                            name=nc.get_next_instruction_name(),
                            engine=inst.engine,
                            bass_nofuse=True,
                            sync_info=mybir.SyncInfo(on_wait=[w], on_update=[]),
                        ))
                    si.on_wait = [si.on_wait[-1]]
                if (isinstance(inst, mybir.InstISA)
                        and inst.isa_opcode == rc_op):
                    struct = inst.ant_dict
                    for semid in range(struct["range_first"],
                                       struct["range_last"] + 1):
                        newlist.append(mybir.InstNoOp(
                            name=nc.get_next_instruction_name(),
                            engine=inst.engine,
                            bass_nofuse=True,
                            sync_info=mybir.SyncInfo(
                                on_wait=list(si.on_wait) if (
                                    si and semid == struct["range_first"]
                                ) else [],
                                on_update=[mybir.SyncUpdate(
                                    sync_type="semaphore", id=semid,
                                    update_mode="sem-wr-imm",
                                    update_value=0)],
                            ),
                        ))
                    continue
                newlist.append(inst)
            blk.instructions[:] = newlist
    return nc


def _numpy_fallback(predictions, weight, targets):
    """Exact reimplementation of the reference in numpy (float32 math)."""
    predictions = np.asarray(predictions, np.float32)
    targets = np.asarray(targets)
    weight = np.asarray(weight, np.float32)
    Bf, Cf, Hf, Wf = predictions.shape
    big = np.float32(Hf + Wf)
    total = np.float64(0.0)
    wn = (weight / weight.sum()).astype(np.float32)
    for b in range(Bf):
        pm = predictions[b] - predictions[b].max(axis=0, keepdims=True)
        ex = np.exp(pm, dtype=np.float32)
        probs = ex / ex.sum(axis=0, keepdims=True)
        for c in range(Cf):
            p = (targets[b] == c)
            notp = ~p
            # 1D row distances with BIG init/clamp (scan along axis 1)
            fwd = np.zeros((Hf, Wf), np.float32)
            st = np.full((Hf,), big, np.float32)
            for t in range(Wf):
                st = np.where(notp[:, t], st + 1.0, 0.0)
                fwd[:, t] = st
            bwd = np.zeros((Hf, Wf), np.float32)
            st = np.full((Hf,), big, np.float32)
            for t in range(Wf - 1, -1, -1):
                st = np.where(notp[:, t], st + 1.0, 0.0)
                bwd[:, t] = st
            g = np.minimum(np.minimum(fwd, bwd), big)
            i = np.arange(Hf, dtype=np.float32)
            A = (i[:, None] - i[None, :]) ** 2
            d2 = (A[:, :, None] + (g * g)[None, :, :]).min(axis=1)
            d = np.sqrt(d2)
            dist = np.where(p, np.float32(-1.0) * d.max(), d)
            total += np.float64((probs[c] * dist).sum(dtype=np.float64)) * wn[c]
    return np.float32(total / (Bf * Cf * Hf * Wf))


def kernel(predictions, weight, targets):
    predictions = np.ascontiguousarray(np.asarray(predictions, np.float32))
    targets = np.ascontiguousarray(np.asarray(targets, np.int32))
    weight = np.asarray(weight, np.float32)

    safe_inputs = (
        np.all(np.isfinite(weight)) and np.all(weight > 0)
        and np.all(np.isfinite(predictions))
        and float(np.abs(predictions).max()) < 80.0
    )
    if not safe_inputs:
        return _numpy_fallback(predictions, weight, targets)

    from concourse.bass_utils import run_bass_kernel_spmd

    if "nc" not in _CACHE:
        _CACHE["nc"] = _build_nc()
    nc = _CACHE["nc"]

    wband = _host_wband()
    in_maps = [
        {
            "predictions": predictions[i * BPC:(i + 1) * BPC],
            "targets": targets[i * BPC:(i + 1) * BPC],
            "wband": wband,
        }
        for i in range(NCORES)
    ]
    res = run_bass_kernel_spmd(nc, in_maps, core_ids=list(range(NCORES)))
    stats = np.stack([r["out_stats"] for r in res.results])  # [8, 128, 32]

    S1 = stats[:, :, 0:10].sum(axis=1, dtype=np.float64).reshape(NCORES, BPC, C)
    S2 = stats[:, :, 10:20].sum(axis=1, dtype=np.float64).reshape(NCORES, BPC, C)
    maxd2 = stats[:, :, 20:30].max(axis=1).reshape(NCORES, BPC, C)

    if maxd2.max() > float(R * R):
        return _numpy_fallback(predictions, weight, targets)

    M = np.sqrt(maxd2.astype(np.float32)).astype(np.float64)
    wn = (weight / weight.sum()).astype(np.float64)
    loss = ((S1 - M * S2) * wn[None, None, :]).sum() / float(B * C * H * W)
    return np.float32(loss)


# revision 31
# speedup vs baseline: 1.0399x; 1.0399x over previous
"""DistanceLoss (EDT + weighted softmax loss) on 8 Trainium2 NeuronCores.

Sharding: data-parallel over batch. Each of the 8 cores processes 2 of the 16
batch samples (all 5 classes). Per (b, c) slice:

  Stage 1 (column distances g): the 1D distance along h is computed with a
  banded MATMUL log-sum trick on the idle PE array:
      S[i,j] = sum_{|s|<=8} 64^{-|s|} z[i+s, j]  (contract over partitions)
  so d_est = -log64(S + 64^-9) lies in (g-0.19, g] and with x = d_est+0.33
  both trunc(x) and round-nearest(x) equal g exactly for g <= 8 (saturating
  to 9 beyond, which preserves the band-check semantics) — one DVE f32->i32
  cast floors g regardless of the convert rounding mode.  Chain per class
  group: PE matmuls -> ACT Ln (from PSUM) -> ACT Identity (scale+bias) ->
  DVE casts -> ACT Square -> G2.

  Stage 2 (banded min-plus along w, radius R=6) in bf16 on DVE.  Classes are
  processed in two groups ({0,1} then {2,3,4}) so the DVE min-plus of group
  0 overlaps the PE/ACT stage-1 chain of group 1 (and of the next sample).

  Softmax pieces on ACT/DVE in bf16; per-class partial sums
  S1 = sum(d*probs), S2 = sum_present(probs) via DVE scalar_tensor_tensor
  accumulators; per-class max(d^2) via one 5-class tensor_reduce.

Host combine: loss = sum_{b,c} w_c/sum(w) * (S1 - sqrt(maxd2)*S2) / N.
Exact whenever true max EDT distance <= R (verified on gathered maxd2;
pure-numpy exact fallback otherwise — never taken for the target inputs).
"""

import math
import numpy as np

B, C, H, W = 16, 5, 256, 256
NCORES = 8
BPC = B // NCORES  # batches per core
R = 6              # min-plus band radius (exact iff max EDT distance <= R)
P = 128
CW = W + 6         # padded chunk width (pads >= R break min-plus chaining)
GBAND = 8          # log-sum band for column distances (exact g <= GBAND)
LSB = 64.0         # log-sum base (spread log_b(2b/(b-1)) ~ 0.17 << 1)
LNB = math.log(LSB)
XBIAS = 0.33       # trunc(d_est + XBIAS) == round(d_est + XBIAS) == g
CGROUPS = ((0,), (1, 2, 3, 4))

_CACHE = {}


def _host_wband():
    """Banded 64^{-|dist|} weight blocks for the column-distance matmul.
    [128, 3, 128] bf16: [:,0,:]=diag block, [:,1,:]=block(hb_in=0->hb_out=1),
    [:,2,:]=block(hb_in=1->hb_out=0). lhsT convention: [k=p_in, m=p_out].
    """
    idx = np.arange(P)
    d_diag = np.abs(idx[:, None] - idx[None, :]).astype(np.float64)
    d_01 = (P + idx[None, :] - idx[:, None]).astype(np.float64)  # |h_out-h_in|
    d_10 = (P + idx[:, None] - idx[None, :]).astype(np.float64)
    blocks = []
    for dm in (d_diag, d_01, d_10):
        w = np.where(dm <= GBAND, LSB ** (-dm), 0.0)
        blocks.append(w)
    out = np.stack(blocks, axis=1).astype(np.float32)  # [128, 3, 128]
    import ml_dtypes
    return out.astype(ml_dtypes.bfloat16)


def _build_nc(legalize=True, race_detect=True, walrus_fixups=True):
    import concourse.bass as bass
    import concourse.mybir as mybir
    import concourse.tile as tile

    f32 = mybir.dt.float32
    i32 = mybir.dt.int32
    bf16 = mybir.dt.bfloat16
    Alu = mybir.AluOpType
    Act = mybir.ActivationFunctionType

    nc = bass.Bass(detect_race_conditions=race_detect)
    pred_d = nc.dram_tensor("predictions", [BPC, C, H, W], f32, kind="ExternalInput")
    tgt_d = nc.dram_tensor("targets", [BPC, H, W], i32, kind="ExternalInput")
    wband_d = nc.dram_tensor("wband", [P, 3, P], bf16, kind="ExternalInput")
    # stats columns: [0:10] S1 (b*5+c), [10:20] S2, [20:30] maxd2, [30:32] pad
    out_d = nc.dram_tensor("out_stats", [P, 32], f32, kind="ExternalOutput")

    with tile.TileContext(nc) as tc:
        with (
            tc.tile_pool(name="const", bufs=1) as cpool,
            tc.tile_pool(name="work", bufs=2) as pool,
            tc.tile_pool(name="dmabuf", bufs=2) as dpool,
            tc.tile_pool(name="psA", bufs=1, space="PSUM") as psA,
            tc.tile_pool(name="psB", bufs=1, space="PSUM") as psB,
            tc.tile_pool(name="psC", bufs=1, space="PSUM") as psC,
        ):
            # hoist all DMAs: targets first (they gate the z/matmul chain)
            t_tiles = []
            pred_tiles = []
            for b in range(BPC):
                t_i32 = dpool.tile([P, 2, W], i32, tag="t_i32")
                nc.sync.dma_start(
                    t_i32[:], tgt_d[b].rearrange("(n p) w -> p n w", p=P))
                t_tiles.append(t_i32)
            wsb = cpool.tile([P, 3, P], bf16)
            nc.sync.dma_start(wsb[:], wband_d[:])
            for b in range(BPC):
                pred = dpool.tile([P, 2, C, W], f32, tag="pred")
                pred_v = pred_d[b].rearrange("c (n p) w -> p n c w", p=P)
                for hb in range(2):
                    nc.sync.dma_start(pred[:, hb], pred_v[:, hb])
                pred_tiles.append(pred)

            stats = cpool.tile([P, 32], f32)
            nc.vector.memset(stats[:], 0.0)
            ones1 = cpool.tile([P, 1], bf16)
            nc.vector.memset(ones1[:], 1.0)

            bias_ln = cpool.tile([P, 1], f32)
            nc.vector.memset(bias_ln[:], LSB ** -9)
            bias_x = cpool.tile([P, 1], f32)
            nc.vector.memset(bias_x[:], XBIAS)

            # warm-up during the DMA cold-start: preload the ln/exp ACT
            # table and give the PE some dummy matmuls to raise its pstate
            warm = cpool.tile([P, 256], bf16)
            nc.vector.memset(warm[:], 1.0)
            wjunk = cpool.tile([P, 64], f32)
            nc.scalar.activation(wjunk[:, :1], bias_ln[:], Act.Ln)
            for k in range(16):
                psW = psC.tile([1, 256], f32, tag=f"psD{k % 2}", name="psW")
                nc.tensor.matmul(psW[0:1, :], ones1[:], warm[:],
                                 start=True, stop=True)

            # ---- per-sample tiles (pool bufs=2 keeps both alive) ----
            zs, e_alls, eqs, G2bs, d2ps, d_alls = [], [], [], [], [], []
            for b in range(BPC):
                t_b = pool.tile([P, 2, W], bf16, tag="t_b")
                nc.vector.tensor_copy(t_b[:], t_tiles[b][:])
                z = pool.tile([P, C, 2, W], bf16, tag="z")
                for c in range(C):
                    nc.vector.tensor_scalar(
                        z[:, c], t_b[:], float(c), None, Alu.is_equal)
                zs.append(z)
                G2b = pool.tile([P, C, 2, CW], bf16, tag="G2b")
                nc.vector.memset(G2b[:, :, :, W:], 1.0e9)
                G2bs.append(G2b)
                d2ps.append(pool.tile([P, C, 2, CW], bf16, tag="d2p", name="d2p"))
                d_alls.append(pool.tile([P, C, 2, W], bf16, tag="d_all", name="d_all"))

            def stage1(b, gi):
                # column distances for class group gi of sample b:
                # banded matmul -> Ln -> scaled int cast -> Square -> G2
                grp = CGROUPS[gi]
                nclass = len(grp)
                c0 = grp[0]
                z = zs[b]
                ps = (psA if gi == 0 else psB)
                S = ps.tile([P, nclass, 2, W], f32, tag=f"S{gi}")
                for ci in range(nclass):
                    for hbo in range(2):
                        nc.tensor.matmul(
                            S[:, ci, hbo, :], wsb[:, 0, :],
                            z[:, grp[ci], hbo, :],
                            start=(hbo == 0), stop=False)
                for ci in range(nclass):
                    nc.tensor.matmul(
                        S[:, ci, 1, :], wsb[:, 1, :], z[:, grp[ci], 0, :],
                        start=False, stop=False)
                for ci in range(nclass):
                    nc.tensor.matmul(
                        S[:, ci, 0, :], wsb[:, 2, :], z[:, grp[ci], 1, :],
                        start=False, stop=True)
                NG = nclass * 2 * W
                u = pool.tile([P, 4, 2, W], f32, tag="u")
                uf = u[:].rearrange("p c a w -> p (c a w)")[:, :NG]
                nc.scalar.activation(
                    uf, S[:].rearrange("p c a w -> p (c a w)"),
                    Act.Ln, bias=bias_ln[:])
                g_i = pool.tile([P, 4, 2, W], i32, tag="g_i")
                gif = g_i[:].rearrange("p c a w -> p (c a w)")[:, :NG]
                nc.scalar.activation(
                    gif, uf, Act.Identity, scale=-1.0 / LNB, bias=bias_x[:])
                nc.scalar.activation(
                    G2bs[b][:, c0:c0 + nclass, :, :W],
                    gif.rearrange("p (c a w) -> p c a w", c=nclass, a=2, w=W),
                    Act.Square)

            def minplus(b, gi):
                # banded min-plus along w on the flat padded group slice
                grp = CGROUPS[gi]
                ncl = len(grp)
                c0 = grp[0]
                TLG = ncl * 2 * CW
                Gf = G2bs[b][:, c0:c0 + ncl].rearrange("p c a w -> p (c a w)")
                Df = d2ps[b][:, c0:c0 + ncl].rearrange("p c a w -> p (c a w)")
                nc.vector.tensor_copy(Df, Gf)
                for dlt in range(1, R + 1):
                    tmp = pool.tile([P, C, 2, CW], bf16, tag="tmp")
                    tmpf = tmp[:].rearrange("p c a w -> p (c a w)")[:, :TLG]
                    nc.vector.tensor_scalar(
                        tmpf, Gf, float(dlt * dlt), None, Alu.add)
                    nc.vector.tensor_tensor(
                        Df[:, dlt:], Df[:, dlt:], tmpf[:, :TLG - dlt],
                        Alu.min)
                    nc.vector.tensor_tensor(
                        Df[:, :TLG - dlt], Df[:, :TLG - dlt],
                        tmpf[:, dlt:], Alu.min)

            def sqrt_grp(b, gis):
                c0 = CGROUPS[gis[0]][0]
                ncl = sum(len(CGROUPS[gi]) for gi in gis)
                nc.scalar.activation(
                    d_alls[b][:, c0:c0 + ncl],
                    d2ps[b][:, c0:c0 + ncl, :, :W], Act.Sqrt)

            def softmax(b):
                # e = exp(pred), q = 1/sum_c e (ACT ln/exp), probs = e*q
                pred = pred_tiles[b]
                e_all = pool.tile([P, C, 2, W], bf16, tag="e_all")
                nc.scalar.activation(
                    e_all[:].rearrange("p c a w -> p a c w"), pred[:], Act.Exp)
                sA = pool.tile([P, 2, W], bf16, tag="sA")
                sB = pool.tile([P, 2, W], bf16, tag="sB")
                nc.vector.tensor_tensor(
                    sA[:], e_all[:, 0], e_all[:, 1], Alu.add)
                nc.vector.tensor_tensor(
                    sB[:], e_all[:, 2], e_all[:, 3], Alu.add)
                nc.vector.tensor_tensor(sA[:], sA[:], sB[:], Alu.add)
                nc.vector.tensor_tensor(sA[:], sA[:], e_all[:, 4], Alu.add)
                lg = pool.tile([P, 2, W], f32, tag="lg")
                nc.scalar.activation(lg[:], sA[:], Act.Ln)
                q = pool.tile([P, 2, W], bf16, tag="q")
                nc.scalar.activation(q[:], lg[:], Act.Exp, scale=-1.0)
                eq = pool.tile([P, C, 2, W], bf16, tag="eq")
                nc.vector.tensor_tensor(
                    eq[:], e_all[:],
                    q[:].unsqueeze(1).broadcast_to([P, C, 2, W]), Alu.mult)
                eqs.append(eq)

            def stats_grp(b, classes):
                # S1/S2 for the given classes: d*eq and z*eq products (DVE),
                # ones-matmul partition reduce (PE), free-axis accumulate
                # into stats row 0 (ACT)
                c0, ncl = classes[0], len(classes)
                eq = eqs[b]
                de = pool.tile([P, C, 2, W], bf16, tag="de")
                nc.vector.tensor_tensor(
                    de[:, c0:c0 + ncl].rearrange("p c a w -> p (c a w)"),
                    d_alls[b][:, c0:c0 + ncl].rearrange("p c a w -> p (c a w)"),
                    eq[:, c0:c0 + ncl].rearrange("p c a w -> p (c a w)"),
                    Alu.mult)
                ze = pool.tile([P, C, 2, W], bf16, tag="ze")
                nc.vector.tensor_tensor(
                    ze[:, c0:c0 + ncl].rearrange("p c a w -> p (c a w)"),
                    zs[b][:, c0:c0 + ncl].rearrange("p c a w -> p (c a w)"),
                    eq[:, c0:c0 + ncl].rearrange("p c a w -> p (c a w)"),
                    Alu.mult)
                junkA = pool.tile([1, 512], f32, tag="junkA")
                pairs = [(de, 0, c) for c in classes] + \
                        [(ze, 10, c) for c in classes]
                for k, (src_t, off, c) in enumerate(pairs):
                    psD = psC.tile([1, 512], f32, tag=f"psD{k % 2}",
                                   name="psD")
                    nc.tensor.matmul(
                        psD[0:1, :], ones1[:],
                        src_t[:, c].rearrange("p a w -> p (a w)"),
                        start=True, stop=True)
                    col = off + b * C + c
                    nc.scalar.activation(
                        junkA[0:1, :], psD[0:1, :], Act.Identity,
                        accum_out=stats[0:1, col:col + 1])

            def stats_dve(b, classes):
                # DVE stt accumulators (used for the final tail group so the
                # PE/ACT pipeline is not the last thing standing)
                eq = eqs[b]
                junk = pool.tile([P, 2, W], bf16, tag="junk")
                for c in classes:
                    c1 = b * C + c
                    nc.vector.scalar_tensor_tensor(
                        junk[:], d_alls[b][:, c], 0.0, eq[:, c],
                        Alu.add, Alu.mult,
                        accum_out=stats[:, c1:c1 + 1])
                    nc.vector.scalar_tensor_tensor(
                        junk[:], zs[b][:, c], 0.0, eq[:, c],
                        Alu.add, Alu.mult,
                        accum_out=stats[:, 10 + c1:11 + c1])

            def maxd2(b):
                nc.vector.tensor_reduce(
                    stats[:, 20 + b * C:20 + (b + 1) * C],
                    d2ps[b][:, :, :, :W], mybir.AxisListType.XY, Alu.max)

            # ---- schedule: interleave stage-1 and min-plus so ACT's
            # Square(G0) lands before anything queues behind it; b0 min-plus
            # overlaps b1 stage-1; the final (small) group's stats run as DVE
            # accumulators so the PE/ACT pipeline is not the tail ----
            stage1(0, 0)
            minplus(0, 0)
            stage1(0, 1)
            minplus(0, 1)
            stage1(1, 1)
            stage1(1, 0)
            softmax(0)
            sqrt_grp(0, (0, 1))
            maxd2(0)
            softmax(1)
            stats_grp(0, (0, 1, 2, 3, 4))
            minplus(1, 1)
            sqrt_grp(1, (1,))
            stats_grp(1, CGROUPS[1])
            minplus(1, 0)
            sqrt_grp(1, (0,))
            maxd2(1)
            stats_dve(1, CGROUPS[0])

            nc.sync.dma_start(out_d[:], stats[:])

    # walrus codegen in this toolchain allows only ONE sync wait per
    # instruction; split extras onto same-engine NoOps inserted right before
    # (engine streams are in-order, so this is semantically identical). It
    # also rejects the EVENT_SEMAPHORE_RANGE_CLEAR encoding; replace it with
    # per-semaphore `sem-wr-imm 0` updates on NoOps.
    if not walrus_fixups:
        return nc
    _walrus_fixups(nc)
    return nc


def _walrus_fixups(nc):
    import concourse.mybir as mybir
    rc_op = nc.isa.Opcode.NEURON_ISA_TPB_OPCODE_EVENT_SEMAPHORE_RANGE_CLEAR.value
    for f in nc.m.functions:
        for blk in f.blocks:
            newlist = []
            for inst in blk.instructions:
                si = inst.sync_info
                if si is not None and si.on_wait and len(si.on_wait) > 1:
                    for w in si.on_wait[:-1]:
                        newlist.append(mybir.InstNoOp(
                            name=nc.get_next_instruction_name(),
                            engine=inst.engine,
                            bass_nofuse=True,
                            sync_info=mybir.SyncInfo(on_wait=[w], on_update=[]),
                        ))
                    si.on_wait = [si.on_wait[-1]]
                if (isinstance(inst, mybir.InstISA)
                        and inst.isa_opcode == rc_op):
                    struct = inst.ant_dict
                    for semid in range(struct["range_first"],
                                       struct["range_last"] + 1):
                        newlist.append(mybir.InstNoOp(
                            name=nc.get_next_instruction_name(),
                            engine=inst.engine,
                            bass_nofuse=True,
                            sync_info=mybir.SyncInfo(
                                on_wait=list(si.on_wait) if (
                                    si and semid == struct["range_first"]
                                ) else [],
                                on_update=[mybir.SyncUpdate(
                                    sync_type="semaphore", id=semid,
                                    update_mode="sem-wr-imm",
                                    update_value=0)],
                            ),
                        ))
                    continue
                newlist.append(inst)
            blk.instructions[:] = newlist
    return nc


def _numpy_fallback(predictions, weight, targets):
    """Exact reimplementation of the reference in numpy (float32 math)."""
    predictions = np.asarray(predictions, np.float32)
    targets = np.asarray(targets)
    weight = np.asarray(weight, np.float32)
    Bf, Cf, Hf, Wf = predictions.shape
    big = np.float32(Hf + Wf)
    total = np.float64(0.0)
    wn = (weight / weight.sum()).astype(np.float32)
    for b in range(Bf):
        pm = predictions[b] - predictions[b].max(axis=0, keepdims=True)
        ex = np.exp(pm, dtype=np.float32)
        probs = ex / ex.sum(axis=0, keepdims=True)
        for c in range(Cf):
            p = (targets[b] == c)
            notp = ~p
            # 1D row distances with BIG init/clamp (scan along axis 1)
            fwd = np.zeros((Hf, Wf), np.float32)
            st = np.full((Hf,), big, np.float32)
            for t in range(Wf):
                st = np.where(notp[:, t], st + 1.0, 0.0)
                fwd[:, t] = st
            bwd = np.zeros((Hf, Wf), np.float32)
            st = np.full((Hf,), big, np.float32)
            for t in range(Wf - 1, -1, -1):
                st = np.where(notp[:, t], st + 1.0, 0.0)
                bwd[:, t] = st
            g = np.minimum(np.minimum(fwd, bwd), big)
            i = np.arange(Hf, dtype=np.float32)
            A = (i[:, None] - i[None, :]) ** 2
            d2 = (A[:, :, None] + (g * g)[None, :, :]).min(axis=1)
            d = np.sqrt(d2)
            dist = np.where(p, np.float32(-1.0) * d.max(), d)
            total += np.float64((probs[c] * dist).sum(dtype=np.float64)) * wn[c]
    return np.float32(total / (Bf * Cf * Hf * Wf))


def kernel(predictions, weight, targets):
    predictions = np.ascontiguousarray(np.asarray(predictions, np.float32))
    targets = np.ascontiguousarray(np.asarray(targets, np.int32))
    weight = np.asarray(weight, np.float32)

    safe_inputs = (
        np.all(np.isfinite(weight)) and np.all(weight > 0)
        and np.all(np.isfinite(predictions))
        and float(np.abs(predictions).max()) < 80.0
    )
    if not safe_inputs:
        return _numpy_fallback(predictions, weight, targets)

    from concourse.bass_utils import run_bass_kernel_spmd

    if "nc" not in _CACHE:
        _CACHE["nc"] = _build_nc()
    nc = _CACHE["nc"]

    wband = _host_wband()
    in_maps = [
        {
            "predictions": predictions[i * BPC:(i + 1) * BPC],
            "targets": targets[i * BPC:(i + 1) * BPC],
            "wband": wband,
        }
        for i in range(NCORES)
    ]
    res = run_bass_kernel_spmd(nc, in_maps, core_ids=list(range(NCORES)))
    stats = np.stack([r["out_stats"] for r in res.results])  # [8, 128, 32]

    S1 = stats[:, :, 0:10].sum(axis=1, dtype=np.float64).reshape(NCORES, BPC, C)
    S2 = stats[:, :, 10:20].sum(axis=1, dtype=np.float64).reshape(NCORES, BPC, C)
    maxd2 = stats[:, :, 20:30].max(axis=1).reshape(NCORES, BPC, C)

    if maxd2.max() > float(R * R):
        return _numpy_fallback(predictions, weight, targets)

    M = np.sqrt(maxd2.astype(np.float32)).astype(np.float64)
    wn = (weight / weight.sum()).astype(np.float64)
    loss = ((S1 - M * S2) * wn[None, None, :]).sum() / float(B * C * H * W)
    return np.float32(loss)


# revision 32
# speedup vs baseline: 1.0526x; 1.0122x over previous
"""DistanceLoss (EDT + weighted softmax loss) on 8 Trainium2 NeuronCores.

Sharding: data-parallel over batch. Each of the 8 cores processes 2 of the 16
batch samples (all 5 classes). Per (b, c) slice:

  Stage 1 (column distances g): the 1D distance along h is computed with a
  banded MATMUL log-sum trick on the idle PE array:
      S[i,j] = sum_{|s|<=8} 64^{-|s|} z[i+s, j]  (contract over partitions)
  so d_est = -log64(S + 64^-9) lies in (g-0.19, g] and with x = d_est+0.33
  both trunc(x) and round-nearest(x) equal g exactly for g <= 8 (saturating
  to 9 beyond, which preserves the band-check semantics) — one DVE f32->i32
  cast floors g regardless of the convert rounding mode.  Chain per class
  group: PE matmuls -> ACT Ln (from PSUM) -> ACT Identity (scale+bias) ->
  DVE casts -> ACT Square -> G2.

  Stage 2 (banded min-plus along w, radius R=6) in bf16 on DVE.  Classes are
  processed in two groups ({0,1} then {2,3,4}) so the DVE min-plus of group
  0 overlaps the PE/ACT stage-1 chain of group 1 (and of the next sample).

  Softmax pieces on ACT/DVE in bf16; per-class partial sums
  S1 = sum(d*probs), S2 = sum_present(probs) via DVE scalar_tensor_tensor
  accumulators; per-class max(d^2) via one 5-class tensor_reduce.

Host combine: loss = sum_{b,c} w_c/sum(w) * (S1 - sqrt(maxd2)*S2) / N.
Exact whenever true max EDT distance <= R (verified on gathered maxd2;
pure-numpy exact fallback otherwise — never taken for the target inputs).
"""

import math
import numpy as np

B, C, H, W = 16, 5, 256, 256
NCORES = 8
BPC = B // NCORES  # batches per core
R = 6              # min-plus band radius (exact iff max EDT distance <= R)
P = 128
CW = W + 6         # padded chunk width (pads >= R break min-plus chaining)
GBAND = 8          # log-sum band for column distances (exact g <= GBAND)
LSB = 64.0         # log-sum base (spread log_b(2b/(b-1)) ~ 0.17 << 1)
LNB = math.log(LSB)
XBIAS = 0.33       # trunc(d_est + XBIAS) == round(d_est + XBIAS) == g
CGROUPS = ((0,), (1, 2), (3, 4))

_CACHE = {}


def _host_wband():
    """Banded 64^{-|dist|} weight blocks for the column-distance matmul.
    [128, 3, 128] bf16: [:,0,:]=diag block, [:,1,:]=block(hb_in=0->hb_out=1),
    [:,2,:]=block(hb_in=1->hb_out=0). lhsT convention: [k=p_in, m=p_out].
    """
    idx = np.arange(P)
    d_diag = np.abs(idx[:, None] - idx[None, :]).astype(np.float64)
    d_01 = (P + idx[None, :] - idx[:, None]).astype(np.float64)  # |h_out-h_in|
    d_10 = (P + idx[:, None] - idx[None, :]).astype(np.float64)
    blocks = []
    for dm in (d_diag, d_01, d_10):
        w = np.where(dm <= GBAND, LSB ** (-dm), 0.0)
        blocks.append(w)
    out = np.stack(blocks, axis=1).astype(np.float32)  # [128, 3, 128]
    import ml_dtypes
    return out.astype(ml_dtypes.bfloat16)


def _build_nc(legalize=True, race_detect=True, walrus_fixups=True):
    import concourse.bass as bass
    import concourse.mybir as mybir
    import concourse.tile as tile

    f32 = mybir.dt.float32
    i32 = mybir.dt.int32
    bf16 = mybir.dt.bfloat16
    Alu = mybir.AluOpType
    Act = mybir.ActivationFunctionType

    nc = bass.Bass(detect_race_conditions=race_detect)
    pred_d = nc.dram_tensor("predictions", [BPC, C, H, W], f32, kind="ExternalInput")
    tgt_d = nc.dram_tensor("targets", [BPC, H, W], i32, kind="ExternalInput")
    wband_d = nc.dram_tensor("wband", [P, 3, P], bf16, kind="ExternalInput")
    # stats columns: [0:10] S1 (b*5+c), [10:20] S2, [20:30] maxd2, [30:32] pad
    out_d = nc.dram_tensor("out_stats", [P, 32], f32, kind="ExternalOutput")

    with tile.TileContext(nc) as tc:
        with (
            tc.tile_pool(name="const", bufs=1) as cpool,
            tc.tile_pool(name="work", bufs=2) as pool,
            tc.tile_pool(name="dmabuf", bufs=2) as dpool,
            tc.tile_pool(name="psA", bufs=1, space="PSUM") as psA,
            tc.tile_pool(name="psB", bufs=1, space="PSUM") as psB,
            tc.tile_pool(name="psC", bufs=1, space="PSUM") as psC,
        ):
            # hoist all DMAs: targets first (they gate the z/matmul chain)
            t_tiles = []
            pred_tiles = []
            for b in range(BPC):
                t_i32 = dpool.tile([P, 2, W], i32, tag="t_i32")
                nc.sync.dma_start(
                    t_i32[:], tgt_d[b].rearrange("(n p) w -> p n w", p=P))
                t_tiles.append(t_i32)
            wsb = cpool.tile([P, 3, P], bf16)
            nc.sync.dma_start(wsb[:], wband_d[:])
            for b in range(BPC):
                pred = dpool.tile([P, 2, C, W], f32, tag="pred")
                pred_v = pred_d[b].rearrange("c (n p) w -> p n c w", p=P)
                for hb in range(2):
                    nc.sync.dma_start(pred[:, hb], pred_v[:, hb])
                pred_tiles.append(pred)

            stats = cpool.tile([P, 32], f32)
            nc.vector.memset(stats[:], 0.0)
            ones1 = cpool.tile([P, 1], bf16)
            nc.vector.memset(ones1[:], 1.0)

            bias_ln = cpool.tile([P, 1], f32)
            nc.vector.memset(bias_ln[:], LSB ** -9)
            bias_x = cpool.tile([P, 1], f32)
            nc.vector.memset(bias_x[:], XBIAS)

            # warm-up during the DMA cold-start: preload the ln/exp ACT
            # table and give the PE some dummy matmuls to raise its pstate
            warm = cpool.tile([P, 256], bf16)
            nc.vector.memset(warm[:], 1.0)
            wjunk = cpool.tile([P, 64], f32)
            nc.scalar.activation(wjunk[:, :1], bias_ln[:], Act.Ln)
            for k in range(16):
                psW = psC.tile([1, 256], f32, tag=f"psD{k % 2}", name="psW")
                nc.tensor.matmul(psW[0:1, :], ones1[:], warm[:],
                                 start=True, stop=True)

            # ---- per-sample tiles (pool bufs=2 keeps both alive) ----
            zs, e_alls, eqs, G2bs, d2ps, d_alls = [], [], [], [], [], []
            for b in range(BPC):
                t_b = pool.tile([P, 2, W], bf16, tag="t_b")
                nc.vector.tensor_copy(t_b[:], t_tiles[b][:])
                z = pool.tile([P, C, 2, W], bf16, tag="z")
                for c in range(C):
                    nc.vector.tensor_scalar(
                        z[:, c], t_b[:], float(c), None, Alu.is_equal)
                zs.append(z)
                G2b = pool.tile([P, C, 2, CW], bf16, tag="G2b")
                nc.vector.memset(G2b[:, :, :, W:], 1.0e9)
                G2bs.append(G2b)
                d2ps.append(pool.tile([P, C, 2, CW], bf16, tag="d2p", name="d2p"))
                d_alls.append(pool.tile([P, C, 2, W], bf16, tag="d_all", name="d_all"))

            def stage1(b, gi):
                # column distances for class group gi of sample b:
                # banded matmul -> Ln -> scaled int cast -> Square -> G2
                grp = CGROUPS[gi]
                nclass = len(grp)
                c0 = grp[0]
                z = zs[b]
                ps = (psA if gi == 0 else psB)
                S = ps.tile([P, nclass, 2, W], f32, tag=f"S{gi}", name="S")
                for ci in range(nclass):
                    for hbo in range(2):
                        nc.tensor.matmul(
                            S[:, ci, hbo, :], wsb[:, 0, :],
                            z[:, grp[ci], hbo, :],
                            start=(hbo == 0), stop=False)
                for ci in range(nclass):
                    nc.tensor.matmul(
                        S[:, ci, 1, :], wsb[:, 1, :], z[:, grp[ci], 0, :],
                        start=False, stop=False)
                for ci in range(nclass):
                    nc.tensor.matmul(
                        S[:, ci, 0, :], wsb[:, 2, :], z[:, grp[ci], 1, :],
                        start=False, stop=True)
                NG = nclass * 2 * W
                u = pool.tile([P, 4, 2, W], f32, tag="u")
                uf = u[:].rearrange("p c a w -> p (c a w)")[:, :NG]
                nc.scalar.activation(
                    uf, S[:].rearrange("p c a w -> p (c a w)"),
                    Act.Ln, bias=bias_ln[:])
                g_i = pool.tile([P, 4, 2, W], i32, tag="g_i")
                gif = g_i[:].rearrange("p c a w -> p (c a w)")[:, :NG]
                nc.scalar.activation(
                    gif, uf, Act.Identity, scale=-1.0 / LNB, bias=bias_x[:])
                nc.scalar.activation(
                    G2bs[b][:, c0:c0 + nclass, :, :W],
                    gif.rearrange("p (c a w) -> p c a w", c=nclass, a=2, w=W),
                    Act.Square)

            def minplus(b, gi):
                # banded min-plus along w on the flat padded group slice
                grp = CGROUPS[gi]
                ncl = len(grp)
                c0 = grp[0]
                TLG = ncl * 2 * CW
                Gf = G2bs[b][:, c0:c0 + ncl].rearrange("p c a w -> p (c a w)")
                Df = d2ps[b][:, c0:c0 + ncl].rearrange("p c a w -> p (c a w)")
                nc.vector.tensor_copy(Df, Gf)
                for dlt in range(1, R + 1):
                    tmp = pool.tile([P, C, 2, CW], bf16, tag="tmp")
                    tmpf = tmp[:].rearrange("p c a w -> p (c a w)")[:, :TLG]
                    nc.vector.tensor_scalar(
                        tmpf, Gf, float(dlt * dlt), None, Alu.add)
                    nc.vector.tensor_tensor(
                        Df[:, dlt:], Df[:, dlt:], tmpf[:, :TLG - dlt],
                        Alu.min)
                    nc.vector.tensor_tensor(
                        Df[:, :TLG - dlt], Df[:, :TLG - dlt],
                        tmpf[:, dlt:], Alu.min)

            def sqrt_grp(b, gis):
                c0 = CGROUPS[gis[0]][0]
                ncl = sum(len(CGROUPS[gi]) for gi in gis)
                nc.scalar.activation(
                    d_alls[b][:, c0:c0 + ncl],
                    d2ps[b][:, c0:c0 + ncl, :, :W], Act.Sqrt)

            def softmax(b):
                # e = exp(pred), q = 1/sum_c e (ACT ln/exp), probs = e*q
                pred = pred_tiles[b]
                e_all = pool.tile([P, C, 2, W], bf16, tag="e_all")
                nc.scalar.activation(
                    e_all[:].rearrange("p c a w -> p a c w"), pred[:], Act.Exp)
                sA = pool.tile([P, 2, W], bf16, tag="sA")
                sB = pool.tile([P, 2, W], bf16, tag="sB")
                nc.vector.tensor_tensor(
                    sA[:], e_all[:, 0], e_all[:, 1], Alu.add)
                nc.vector.tensor_tensor(
                    sB[:], e_all[:, 2], e_all[:, 3], Alu.add)
                nc.vector.tensor_tensor(sA[:], sA[:], sB[:], Alu.add)
                nc.vector.tensor_tensor(sA[:], sA[:], e_all[:, 4], Alu.add)
                lg = pool.tile([P, 2, W], f32, tag="lg")
                nc.scalar.activation(lg[:], sA[:], Act.Ln)
                q = pool.tile([P, 2, W], bf16, tag="q")
                nc.scalar.activation(q[:], lg[:], Act.Exp, scale=-1.0)
                eq = pool.tile([P, C, 2, W], bf16, tag="eq")
                nc.vector.tensor_tensor(
                    eq[:], e_all[:],
                    q[:].unsqueeze(1).broadcast_to([P, C, 2, W]), Alu.mult)
                eqs.append(eq)

            def stats_grp(b, classes):
                # S1/S2 for the given classes: d*eq and z*eq products (DVE),
                # ones-matmul partition reduce (PE), free-axis accumulate
                # into stats row 0 (ACT)
                c0, ncl = classes[0], len(classes)
                eq = eqs[b]
                de = pool.tile([P, C, 2, W], bf16, tag="de")
                nc.vector.tensor_tensor(
                    de[:, c0:c0 + ncl].rearrange("p c a w -> p (c a w)"),
                    d_alls[b][:, c0:c0 + ncl].rearrange("p c a w -> p (c a w)"),
                    eq[:, c0:c0 + ncl].rearrange("p c a w -> p (c a w)"),
                    Alu.mult)
                ze = pool.tile([P, C, 2, W], bf16, tag="ze")
                nc.vector.tensor_tensor(
                    ze[:, c0:c0 + ncl].rearrange("p c a w -> p (c a w)"),
                    zs[b][:, c0:c0 + ncl].rearrange("p c a w -> p (c a w)"),
                    eq[:, c0:c0 + ncl].rearrange("p c a w -> p (c a w)"),
                    Alu.mult)
                junkA = pool.tile([1, 512], f32, tag="junkA")
                pairs = [(de, 0, c) for c in classes] + \
                        [(ze, 10, c) for c in classes]
                for k, (src_t, off, c) in enumerate(pairs):
                    psD = psC.tile([1, 512], f32, tag=f"psD{k % 2}",
                                   name="psD")
                    nc.tensor.matmul(
                        psD[0:1, :], ones1[:],
                        src_t[:, c].rearrange("p a w -> p (a w)"),
                        start=True, stop=True)
                    col = off + b * C + c
                    nc.scalar.activation(
                        junkA[0:1, :], psD[0:1, :], Act.Identity,
                        accum_out=stats[0:1, col:col + 1])

            def stats_dve(b, classes):
                # DVE stt accumulators (used for the final tail group so the
                # PE/ACT pipeline is not the last thing standing)
                eq = eqs[b]
                junk = pool.tile([P, 2, W], bf16, tag="junk")
                for c in classes:
                    c1 = b * C + c
                    nc.vector.scalar_tensor_tensor(
                        junk[:], d_alls[b][:, c], 0.0, eq[:, c],
                        Alu.add, Alu.mult,
                        accum_out=stats[:, c1:c1 + 1])
                    nc.vector.scalar_tensor_tensor(
                        junk[:], zs[b][:, c], 0.0, eq[:, c],
                        Alu.add, Alu.mult,
                        accum_out=stats[:, 10 + c1:11 + c1])

            def maxd2(b):
                nc.vector.tensor_reduce(
                    stats[:, 20 + b * C:20 + (b + 1) * C],
                    d2ps[b][:, :, :, :W], mybir.AxisListType.XY, Alu.max)

            # ---- schedule: interleave stage-1 and min-plus so ACT's
            # Square(G0) lands before anything queues behind it; b0 min-plus
            # overlaps b1 stage-1; the final (small) group's stats run as DVE
            # accumulators so the PE/ACT pipeline is not the tail ----
            stage1(0, 0)
            minplus(0, 0)
            stage1(0, 1)
            minplus(0, 1)
            stage1(0, 2)
            minplus(0, 2)
            stage1(1, 2)
            stage1(1, 1)
            stage1(1, 0)
            softmax(0)
            sqrt_grp(0, (0, 1, 2))
            maxd2(0)
            softmax(1)
            stats_grp(0, (0, 1, 2, 3, 4))
            minplus(1, 2)
            sqrt_grp(1, (2,))
            stats_grp(1, CGROUPS[2])
            minplus(1, 1)
            sqrt_grp(1, (1,))
            stats_grp(1, CGROUPS[1])
            minplus(1, 0)
            sqrt_grp(1, (0,))
            maxd2(1)
            stats_dve(1, CGROUPS[0])

            nc.sync.dma_start(out_d[:], stats[:])

    # walrus codegen in this toolchain allows only ONE sync wait per
    # instruction; split extras onto same-engine NoOps inserted right before
    # (engine streams are in-order, so this is semantically identical). It
    # also rejects the EVENT_SEMAPHORE_RANGE_CLEAR encoding; replace it with
    # per-semaphore `sem-wr-imm 0` updates on NoOps.
    if not walrus_fixups:
        return nc
    _walrus_fixups(nc)
    return nc


def _walrus_fixups(nc):
    import concourse.mybir as mybir
    rc_op = nc.isa.Opcode.NEURON_ISA_TPB_OPCODE_EVENT_SEMAPHORE_RANGE_CLEAR.value
    for f in nc.m.functions:
        for blk in f.blocks:
            newlist = []
            for inst in blk.instructions:
                si = inst.sync_info
                if si is not None and si.on_wait and len(si.on_wait) > 1:
                    for w in si.on_wait[:-1]:
                        newlist.append(mybir.InstNoOp(
                            name=nc.get_next_instruction_name(),
                            engine=inst.engine,
                            bass_nofuse=True,
                            sync_info=mybir.SyncInfo(on_wait=[w], on_update=[]),
                        ))
                    si.on_wait = [si.on_wait[-1]]
                if (isinstance(inst, mybir.InstISA)
                        and inst.isa_opcode == rc_op):
                    struct = inst.ant_dict
                    for semid in range(struct["range_first"],
                                       struct["range_last"] + 1):
                        newlist.append(mybir.InstNoOp(
                            name=nc.get_next_instruction_name(),
                            engine=inst.engine,
                            bass_nofuse=True,
                            sync_info=mybir.SyncInfo(
                                on_wait=list(si.on_wait) if (
                                    si and semid == struct["range_first"]
                                ) else [],
                                on_update=[mybir.SyncUpdate(
                                    sync_type="semaphore", id=semid,
                                    update_mode="sem-wr-imm",
                                    update_value=0)],
                            ),
                        ))
                    continue
                newlist.append(inst)
            blk.instructions[:] = newlist
    return nc


def _numpy_fallback(predictions, weight, targets):
    """Exact reimplementation of the reference in numpy (float32 math)."""
    predictions = np.asarray(predictions, np.float32)
    targets = np.asarray(targets)
    weight = np.asarray(weight, np.float32)
    Bf, Cf, Hf, Wf = predictions.shape
    big = np.float32(Hf + Wf)
    total = np.float64(0.0)
    wn = (weight / weight.sum()).astype(np.float32)
    for b in range(Bf):
        pm = predictions[b] - predictions[b].max(axis=0, keepdims=True)
        ex = np.exp(pm, dtype=np.float32)
        probs = ex / ex.sum(axis=0, keepdims=True)
        for c in range(Cf):
            p = (targets[b] == c)
            notp = ~p
            # 1D row distances with BIG init/clamp (scan along axis 1)
            fwd = np.zeros((Hf, Wf), np.float32)
            st = np.full((Hf,), big, np.float32)
            for t in range(Wf):
                st = np.where(notp[:, t], st + 1.0, 0.0)
                fwd[:, t] = st
            bwd = np.zeros((Hf, Wf), np.float32)
            st = np.full((Hf,), big, np.float32)
            for t in range(Wf - 1, -1, -1):
                st = np.where(notp[:, t], st + 1.0, 0.0)
                bwd[:, t] = st
            g = np.minimum(np.minimum(fwd, bwd), big)
            i = np.arange(Hf, dtype=np.float32)
            A = (i[:, None] - i[None, :]) ** 2
            d2 = (A[:, :, None] + (g * g)[None, :, :]).min(axis=1)
            d = np.sqrt(d2)
            dist = np.where(p, np.float32(-1.0) * d.max(), d)
            total += np.float64((probs[c] * dist).sum(dtype=np.float64)) * wn[c]
    return np.float32(total / (Bf * Cf * Hf * Wf))


def kernel(predictions, weight, targets):
    predictions = np.ascontiguousarray(np.asarray(predictions, np.float32))
    targets = np.ascontiguousarray(np.asarray(targets, np.int32))
    weight = np.asarray(weight, np.float32)

    safe_inputs = (
        np.all(np.isfinite(weight)) and np.all(weight > 0)
        and np.all(np.isfinite(predictions))
        and float(np.abs(predictions).max()) < 80.0
    )
    if not safe_inputs:
        return _numpy_fallback(predictions, weight, targets)

    from concourse.bass_utils import run_bass_kernel_spmd

    if "nc" not in _CACHE:
        _CACHE["nc"] = _build_nc()
    nc = _CACHE["nc"]

    wband = _host_wband()
    in_maps = [
        {
            "predictions": predictions[i * BPC:(i + 1) * BPC],
            "targets": targets[i * BPC:(i + 1) * BPC],
            "wband": wband,
        }
        for i in range(NCORES)
    ]
    res = run_bass_kernel_spmd(nc, in_maps, core_ids=list(range(NCORES)))
    stats = np.stack([r["out_stats"] for r in res.results])  # [8, 128, 32]

    S1 = stats[:, :, 0:10].sum(axis=1, dtype=np.float64).reshape(NCORES, BPC, C)
    S2 = stats[:, :, 10:20].sum(axis=1, dtype=np.float64).reshape(NCORES, BPC, C)
    maxd2 = stats[:, :, 20:30].max(axis=1).reshape(NCORES, BPC, C)

    if maxd2.max() > float(R * R):
        return _numpy_fallback(predictions, weight, targets)

    M = np.sqrt(maxd2.astype(np.float32)).astype(np.float64)
    wn = (weight / weight.sum()).astype(np.float64)
    loss = ((S1 - M * S2) * wn[None, None, :]).sum() / float(B * C * H * W)
    return np.float32(loss)
